# revision 29
# baseline (speedup 1.0000x reference)
"""Complex 3D+temporal conv (ComplexPadConv3Dt) on 8 Trainium2 NeuronCores.

Strategy (hardcoded for B=2, T=8, Z=20, Y=64, X=64, C=2, F1=F=32, k=3):
 - Pure data-parallel sharding: 8 cores = B(2) x X-quarters(4). Each core
   computes its (b, 16-wide x slab) including halo; no collectives.
 - Host: weight projection, symmetric padding, channel-separated relayout
   with a (dz:2, dy:3)-shifted 6x partition stack (z-major for 13.8KB
   contiguous DMA runs), final channel-major gather/relayout.
 - All matmuls in bf16 (f32 PSUM accumulation, ~4.8e-3 end-to-end err):
   halves input DMA and keeps K=64 temporal matmuls at 1 cycle/column.
 - Row AND col tile_position packing (verified correct for bf16 on HW):
   spatial conv runs 8 concurrent 24/12x64 tiles (4 row groups x 2 col
   groups = all 8 t-slices per wave, LDWEIGHTS hidden by the rotation);
   temporal conv runs 4 concurrent 64x64 tiles (2 t-groups x 2
   z-parities). Two accumulation groups on disjoint partition halves of
   one PSUM bank may interleave (HW-probed; the sim's partition-blind
   group check is skipped), but a group's taps must all use the SAME PE
   tile row - changing tile_position mid-group crashes the HW, so
   boundary temporal groups are split by row and summed at evacuation.
 - Spatial conv: contraction K=24 = (dz:2, dy:3, c:2, re/im:2) plus a K=12
   wave reading the dz=0 block at a z+2 free offset; 6 accumulating waves
   cover all 27 taps.
 - Temporal conv: K=64 channel contraction, boundary taps folded into 5
   host-precomputed weight variants.
 - No on-device transpose: output is written channel-major as
   [4, NZB, 128, 4096] bf16 blocks (8KB contiguous per partition per DMA,
   host upcasts to f32) and relaid out on the host.
"""

import numpy as np
import ml_dtypes

import concourse.bass as bass
import concourse.bacc as bacc
import concourse.mybir as mybir
from concourse import tile
from concourse.bass_utils import run_bass_kernel_spmd

# Problem constants
B, T, Z, Y, X, C = 2, 8, 20, 64, 64, 2
F1, F = 32, 32
KZ = KY = KX = 3
KT = 3

# Sharding / tiling
XC = 16          # output x columns per core
NXC = X // XC    # 4 x-chunks
XI = XC + 2      # input x columns per core (halo)
ZB = 4           # z rows per block
NZB = Z // ZB    # 5 blocks
ZI = ZB + 2      # slab z extent per block (dz reads need +2)
YP, ZP = Y + 2, Z + 2

F32 = mybir.dt.float32
BF16 = mybir.dt.bfloat16
BF16_NP = ml_dtypes.bfloat16

_NC_CACHE = {}


def _project(wr, wi, zero_mean):
    wr = wr.astype(np.float64)
    wi = wi.astype(np.float64)
    ax = (0, 1, 2, 3)
    if zero_mean:
        wr = wr - wr.mean(ax, keepdims=True)
        wi = wi - wi.mean(ax, keepdims=True)
    norm = np.sqrt((wr * wr + wi * wi).sum(ax, keepdims=True))
    s = 1.0 / np.maximum(norm, 1.0)
    return wr * s, wi * s


def _spatial_lhsT(wsr, wsi):
    """[128, 6*64] bf16. col block w = dx*2 + grp.

    grp=0: rows 32g + dz*12 + dy*4 + c*2 + part (dz in {0,1}), K=24
    grp=1: rows 32g + dy*4 + c*2 + part (the dz=2 tap), K=12
    """
    w = np.zeros((32, 6 * 64), np.float64)
    for dx in range(KX):
        for grp in range(2):
            wcol = (dx * 2 + grp) * 64
            dzs = (0, 1) if grp == 0 else (2,)
            for dzi, dz in enumerate(dzs):
                for dy in range(KY):
                    for c in range(C):
                        rr = dzi * 12 + dy * 4 + c * 2 + 0
                        ri = dzi * 12 + dy * 4 + c * 2 + 1
                        w[rr, wcol + 0:wcol + 32] = wsr[dz, dy, dx, c, :]
                        w[rr, wcol + 32:wcol + 64] = wsi[dz, dy, dx, c, :]
                        w[ri, wcol + 0:wcol + 32] = -wsi[dz, dy, dx, c, :]
                        w[ri, wcol + 32:wcol + 64] = wsr[dz, dy, dx, c, :]
    out = np.zeros((128, 6 * 64), np.float32)
    for g in range(4):
        out[32 * g:32 * g + 32] = w
    return out.astype(BF16_NP)


def _temporal_lhsT(wtr, wti):
    """[128, 5*64] bf16. rows 64d + q*32 + f1 (q=0 spr, 1 spi);
    cols v*64 + part'*32 + f.

    variants v: [wt0, wt1, wt2, wt0+wt1, wt1+wt2]
    """
    wtr = wtr.reshape(KT, F1, F)
    wti = wti.reshape(KT, F1, F)
    variants = [
        (wtr[0], wti[0]),
        (wtr[1], wti[1]),
        (wtr[2], wti[2]),
        (wtr[0] + wtr[1], wti[0] + wti[1]),
        (wtr[1] + wtr[2], wti[1] + wti[2]),
    ]
    w = np.zeros((64, 5 * 64), np.float64)
    for v, (vr, vi) in enumerate(variants):
        w[0:32, v * 64 + 0:v * 64 + 32] = vr          # spr -> yr
        w[0:32, v * 64 + 32:v * 64 + 64] = vi         # spr -> yi
        w[32:64, v * 64 + 0:v * 64 + 32] = -vi        # spi -> yr
        w[32:64, v * 64 + 32:v * 64 + 64] = vr        # spi -> yi
    out = np.zeros((128, 5 * 64), np.float32)
    out[0:64] = w
    out[64:128] = w
    return out.astype(BF16_NP)


def _temporal_taps(t):
    if t == 0:
        return [(0, 3), (1, 2)]
    if t == T - 1:
        return [(T - 2, 0), (T - 1, 4)]
    return [(t - 1, 0), (t, 1), (t + 1, 2)]


def build_program():
    nc = bacc.Bacc(None, target_bir_lowering=False)

    xin = nc.declare_dram_parameter("xin", [24, T, ZP, XI, Y], BF16, isOutput=False)
    wsp = nc.declare_dram_parameter("wsp", [128, 6 * 64], BF16, isOutput=False)
    wtp = nc.declare_dram_parameter("wtp", [128, 5 * 64], BF16, isOutput=False)
    # out_c[tp, zb, p, fo]: p = zpar*64 + q*32 + f (q=0 yr, 1 yi);
    # fo = r*2048 + zp*1024 + d*512 + x*32 + yy; t = tp + 4r;
    # z = zb*4 + zp*2 + zpar; y = d*32 + yy. bf16: host upcasts.
    out_c = nc.declare_dram_parameter("out_c", [4, NZB, 128, 4096], BF16, isOutput=True)

    with tile.TileContext(nc) as tc:
        with (
            tc.tile_pool(name="wpool", bufs=1) as wpool,
            tc.tile_pool(name="slabs", bufs=4) as slab_pool,
            tc.tile_pool(name="spairs", bufs=8) as spair_pool,
            tc.tile_pool(name="stage", bufs=6) as stage_pool,
            tc.tile_pool(name="psum", bufs=8, space="PSUM") as psum_pool,
        ):
            wsp_sb = wpool.tile([128, 6 * 64], BF16, name="wsp_sb", tag="wsp")
            wtp_sb = wpool.tile([128, 5 * 64], BF16, name="wtp_sb", tag="wtp")
            nc.sync.dma_start(out=wsp_sb[:], in_=wsp[:])
            nc.sync.dma_start(out=wtp_sb[:], in_=wtp[:])



            for zb in range(NZB):
                z0 = zb * ZB

                # ---- spatial phase: all 8 t-slices per wave (row+col
                # tiles: 4 row groups x 2 col groups of 24/12x64).
                slabs = []
                for quad in range(2):
                    slab = slab_pool.tile([128, ZI * XI * Y], BF16, name="slab", tag="slab")
                    slab_v = slab.rearrange("p (z x y) -> p z x y", z=ZI, x=XI, y=Y)
                    for g in range(4):
                        t = quad * 4 + g
                        # split by z so the first waves can start before the
                        # whole slab lands
                        nc.sync.dma_start(
                            out=slab_v[32 * g:32 * g + 24, 0:3],
                            in_=xin[:, t, z0:z0 + 3],
                        )
                        nc.sync.dma_start(
                            out=slab_v[32 * g:32 * g + 24, 3:ZI],
                            in_=xin[:, t, z0 + 3:z0 + ZI],
                        )
                    slabs.append(slab_v)

                spairs = []
                for g in range(4):
                    spairs.append(
                        spair_pool.tile([128, ZB * 2 * 512], BF16, name="sp", tag="sp")
                    )

                for z in range(ZB):
                    for j in range(2):
                        banks = []
                        for g in range(4):
                            banks.append(
                                psum_pool.tile([128, 512], F32, name="ps", tag="ps")
                            )
                        # Wave-rotated issue across all 8 tiles so LDWEIGHTS
                        # hits a different tile than the in-flight matmul
                        # (background weight buffer) and all tiles stream
                        # concurrently. The two col groups of a bank live on
                        # disjoint partition halves; interleaving their
                        # accumulation groups is HW-safe (probed), only the
                        # sim's partition-blind group check minds — skipped.
                        for w in range(6):
                            dx, grp = w // 2, w % 2
                            kk = 24 if grp == 0 else 12
                            zoff = z if grp == 0 else z + 2
                            for g in range(4):
                                for col in range(2):
                                    nc.tensor.matmul(
                                        out=banks[g][64 * col:64 * col + 64, :],
                                        lhsT=wsp_sb[
                                            32 * g:32 * g + kk,
                                            w * 64:(w + 1) * 64,
                                        ],
                                        rhs=slabs[col][
                                            32 * g:32 * g + kk,
                                            zoff,
                                            dx:dx + XC,
                                            32 * j:32 * j + 32,
                                        ],
                                        start=(w == 0),
                                        stop=(w == 5),
                                        tile_position=(32 * g, 64 * col),
                                        skip_group_check=True,
                                    )
                        # spair[g] partitions: (t//4)*64 + q*32 + f
                        # free: (z*2 + j)*512 + x*32 + yy
                        fo = (z * 2 + j) * 512
                        for g in range(4):
                            dst = spairs[g][:, fo:fo + 512]
                            if g < 2:
                                nc.scalar.copy(dst, banks[g][:])
                            else:
                                nc.vector.tensor_copy(dst, banks[g][:])

                # ---- temporal phase: 4 concurrent tiles (2 t-groups x 2
                # zpar). zp outer so the zp=0 half only depends on spatial
                # z in {0,1} and can overlap the spatial tail of this zb.
                stages4 = [
                    stage_pool.tile([128, 4096], BF16, name="stg", tag="stg")
                    for _ in range(4)
                ]
                for zp in range(2):
                    for tp in range(4):
                        stage = stages4[tp]
                        plans = []
                        allsets = []
                        for r in range(2):
                            t = tp + 4 * r
                            taps = _temporal_taps(t)
                            # A group's taps must all use the same PE tile row
                            # (changing tile_position mid-accumulation-group
                            # crashes the HW), so split taps by source row and
                            # give each row-run its own bank, summed at evac.
                            runs = []
                            for (s, v) in taps:
                                sr = 64 * (s // 4)
                                if runs and runs[-1][0] == sr:
                                    runs[-1][1].append((s, v))
                                else:
                                    runs.append((sr, [(s, v)]))
                            bsets = []
                            for _ in runs:
                                bsets.append([
                                    psum_pool.tile([128, 512], F32, name="ps", tag="ps")
                                    for _ in range(2)
                                ])
                            plan = []
                            for bidx, (sr, rtaps) in enumerate(runs):
                                for a, (s, v) in enumerate(rtaps):
                                    plan.append((sr, s, v, a == 0,
                                                 a == len(rtaps) - 1, bidx))
                            plans.append(plan)
                            allsets.append(bsets)
                        # Interleave the t-group/d/zpar streams so consecutive
                        # matmuls hit different PE tiles and stream
                        # concurrently (accumulation groups on disjoint
                        # partition halves of a bank may interleave —
                        # HW-probed; only the sim's partition-blind group
                        # check minds, hence skip_group_check).
                        for a in range(max(len(p) for p in plans)):
                            for r in range(2):
                                if a >= len(plans[r]):
                                    continue
                                sr, s, v, first, last, bidx = plans[r][a]
                                sp = spairs[s % 4]
                                for d in range(2):
                                    for zpar in range(2):
                                        fo = ((2 * zp + zpar) * 2) * 512
                                        nc.tensor.matmul(
                                            out=allsets[r][bidx][d][
                                                64 * zpar:64 * zpar + 64, :
                                            ],
                                            lhsT=wtp_sb[
                                                sr:sr + 64,
                                                v * 64:(v + 1) * 64,
                                            ],
                                            rhs=sp[
                                                sr:sr + 64,
                                                fo + d * 512:fo + (d + 1) * 512,
                                            ],
                                            start=first,
                                            stop=last,
                                            tile_position=(sr, 64 * zpar),
                                            skip_group_check=True,
                                        )
                        for r in range(2):
                            bsets = allsets[r]
                            for d in range(2):
                                fo2 = r * 2048 + zp * 1024 + d * 512
                                dst = stage[:, fo2:fo2 + 512]
                                if len(bsets) == 2:
                                    # TensorTensor reads at most one PSUM
                                    # operand: stage the short run via ACT,
                                    # then add the main bank in-place on DVE.
                                    nc.scalar.copy(dst, bsets[1][d][:])
                                    nc.vector.tensor_add(
                                        dst, bsets[0][d][:], dst
                                    )
                                elif d == 0:
                                    nc.scalar.copy(dst, bsets[0][d][:])
                                else:
                                    nc.vector.tensor_copy(dst, bsets[0][d][:])
                for tp in range(4):
                    stage = stages4[tp]
                    # split across partition halves -> two DMA queues
                    nc.sync.dma_start(out=out_c[tp, zb, 0:64], in_=stage[0:64, :])
                    nc.sync.dma_start(out=out_c[tp, zb, 64:128], in_=stage[64:128, :])

    nc.finalize()
    return nc


def _prep_inputs(xr, xi, wxyz_r, wxyz_i, wt_r, wt_i):
    xr = np.asarray(xr, np.float32)
    xi = np.asarray(xi, np.float32)

    wsr, wsi = _project(np.asarray(wxyz_r, np.float64), np.asarray(wxyz_i, np.float64), True)
    wtr, wti = _project(np.asarray(wt_r, np.float64), np.asarray(wt_i, np.float64), False)
    wsp = _spatial_lhsT(wsr, wsi)
    wtp = _temporal_lhsT(wtr, wti)

    pads = [(0, 0), (0, 0), (1, 1), (1, 1), (1, 1), (0, 0)]
    xp = np.stack([np.pad(xr, pads, mode="symmetric"),
                   np.pad(xi, pads, mode="symmetric")])  # [2, B, T, ZP, YP, XP]
    xp = xp.astype(BF16_NP)
    in_maps = []
    for core in range(8):
        b, cx = divmod(core, NXC)
        xs = xp[:, b, :, :, :, XC * cx:XC * cx + XI, :]   # [2, T, ZP, YP, XI, C]
        blocks = []
        for dz in (0, 1):
            zi = np.minimum(np.arange(ZP) + dz, ZP - 1)
            zs = xs[:, :, zi]
            ys = np.stack([zs[:, :, :, dy:dy + Y] for dy in range(KY)], axis=1)
            blocks.append(ys)                       # [2, 3, T, ZP, Y, XI, C]
        bl = np.stack(blocks, axis=1)               # [part, dz, dy, T, ZP, Y, XI, C]
        bl = bl.transpose(1, 2, 7, 0, 3, 4, 6, 5)   # [dz, dy, c, part, T, ZP, XI, Y]
        xin = np.ascontiguousarray(bl.reshape(24, T, ZP, XI, Y))
        in_maps.append({"xin": xin, "wsp": wsp, "wtp": wtp})
    return in_maps


def kernel(xr, xi, wxyz_r, wxyz_i, wt_r, wt_i):
    if "nc" not in _NC_CACHE:
        _NC_CACHE["nc"] = build_program()
    nc = _NC_CACHE["nc"]

    in_maps = _prep_inputs(xr, xi, wxyz_r, wxyz_i, wt_r, wt_i)
    res = run_bass_kernel_spmd(nc, in_maps, list(range(8)))

    yr = np.empty((B, T, Z, Y, X, F), np.float32)
    yi = np.empty((B, T, Z, Y, X, F), np.float32)
    for core in range(8):
        b, cx = divmod(core, NXC)
        oc = np.asarray(res.results[core]["out_c"]).astype(np.float32)
        # [tp, zb, zpar(2), q(2), f(32), r(2), zp(2), d(2), x(16), yy(32)]
        arr = oc.reshape(4, NZB, 2, 2, 32, 2, 2, 2, 16, 32)
        # -> [r, tp, zb, zp, zpar, d, yy, x, f, q]; t = 4r + tp
        arr = arr.transpose(5, 0, 1, 6, 2, 7, 9, 8, 4, 3)
        yr[b, :, :, :, XC * cx:XC * cx + XC, :] = (
            arr[..., 0].reshape(T, Z, Y, XC, F)
        )
        yi[b, :, :, :, XC * cx:XC * cx + XC, :] = (
            arr[..., 1].reshape(T, Z, Y, XC, F)
        )
    return yr, yi


# revision 31
# speedup vs baseline: 1.0647x; 1.0647x over previous
"""Complex 3D+temporal conv (ComplexPadConv3Dt) on 8 Trainium2 NeuronCores.

Strategy (hardcoded for B=2, T=8, Z=20, Y=64, X=64, C=2, F1=F=32, k=3):
 - Pure data-parallel sharding: 8 cores = B(2) x X-quarters(4). Each core
   computes its (b, 16-wide x slab) including halo; no collectives.
 - Host: weight projection, symmetric padding, channel-separated relayout
   with a (dz:2, dy:3)-shifted 6x partition stack (z-major for 13.8KB
   contiguous DMA runs), final channel-major gather/relayout.
 - All matmuls in bf16 (f32 PSUM accumulation, ~4.8e-3 end-to-end err):
   halves input DMA and keeps K=64 temporal matmuls at 1 cycle/column.
 - Row AND col tile_position packing (verified correct for bf16 on HW):
   spatial conv runs 8 concurrent 24/12x64 tiles (4 row groups x 2 col
   groups = all 8 t-slices per wave, LDWEIGHTS hidden by the rotation);
   temporal conv runs 4 concurrent 64x64 tiles (2 t-groups x 2
   z-parities). Two accumulation groups on disjoint partition halves of
   one PSUM bank may interleave (HW-probed; the sim's partition-blind
   group check is skipped), but a group's taps must all use the SAME PE
   tile row - changing tile_position mid-group crashes the HW, so
   boundary temporal groups are split by row and summed at evacuation.
 - Spatial conv: contraction K=24 = (dz:2, dy:3, c:2, re/im:2) plus a K=12
   wave reading the dz=0 block at a z+2 free offset; 6 accumulating waves
   cover all 27 taps.
 - Temporal conv: K=64 channel contraction, boundary taps folded into 5
   host-precomputed weight variants.
 - No on-device transpose: output is written channel-major as
   [4, NZB, 128, 4096] bf16 blocks (8KB contiguous per partition per DMA,
   host upcasts to f32) and relaid out on the host.
"""

import numpy as np
import ml_dtypes

import concourse.bass as bass
import concourse.bacc as bacc
import concourse.mybir as mybir
from concourse import tile
from concourse.bass_utils import run_bass_kernel_spmd

# Problem constants
B, T, Z, Y, X, C = 2, 8, 20, 64, 64, 2
F1, F = 32, 32
KZ = KY = KX = 3
KT = 3

# Sharding / tiling
XC = 16          # output x columns per core
NXC = X // XC    # 4 x-chunks
XI = XC + 2      # input x columns per core (halo)
ZB = 4           # z rows per block
NZB = Z // ZB    # 5 blocks
ZI = ZB + 2      # slab z extent per block (dz reads need +2)
YP, ZP = Y + 2, Z + 2

F32 = mybir.dt.float32
BF16 = mybir.dt.bfloat16
BF16_NP = ml_dtypes.bfloat16

_NC_CACHE = {}


def _project(wr, wi, zero_mean):
    wr = wr.astype(np.float64)
    wi = wi.astype(np.float64)
    ax = (0, 1, 2, 3)
    if zero_mean:
        wr = wr - wr.mean(ax, keepdims=True)
        wi = wi - wi.mean(ax, keepdims=True)
    norm = np.sqrt((wr * wr + wi * wi).sum(ax, keepdims=True))
    s = 1.0 / np.maximum(norm, 1.0)
    return wr * s, wi * s


def _spatial_lhsT(wsr, wsi):
    """[128, 6*64] bf16. col block w = dx*2 + grp.

    grp=0: rows 32g + dz*12 + dy*4 + c*2 + part (dz in {0,1}), K=24
    grp=1: rows 32g + dy*4 + c*2 + part (the dz=2 tap), K=12
    """
    w = np.zeros((32, 6 * 64), np.float64)
    for dx in range(KX):
        for grp in range(2):
            wcol = (dx * 2 + grp) * 64
            dzs = (0, 1) if grp == 0 else (2,)
            for dzi, dz in enumerate(dzs):
                for dy in range(KY):
                    for c in range(C):
                        rr = dzi * 12 + dy * 4 + c * 2 + 0
                        ri = dzi * 12 + dy * 4 + c * 2 + 1
                        w[rr, wcol + 0:wcol + 32] = wsr[dz, dy, dx, c, :]
                        w[rr, wcol + 32:wcol + 64] = wsi[dz, dy, dx, c, :]
                        w[ri, wcol + 0:wcol + 32] = -wsi[dz, dy, dx, c, :]
                        w[ri, wcol + 32:wcol + 64] = wsr[dz, dy, dx, c, :]
    out = np.zeros((128, 6 * 64), np.float32)
    for g in range(4):
        out[32 * g:32 * g + 32] = w
    return out.astype(BF16_NP)


def _temporal_lhsT(wtr, wti):
    """[128, 5*64] bf16. rows 64d + q*32 + f1 (q=0 spr, 1 spi);
    cols v*64 + part'*32 + f.

    variants v: [wt0, wt1, wt2, wt0+wt1, wt1+wt2]
    """
    wtr = wtr.reshape(KT, F1, F)
    wti = wti.reshape(KT, F1, F)
    variants = [
        (wtr[0], wti[0]),
        (wtr[1], wti[1]),
        (wtr[2], wti[2]),
        (wtr[0] + wtr[1], wti[0] + wti[1]),
        (wtr[1] + wtr[2], wti[1] + wti[2]),
    ]
    w = np.zeros((64, 5 * 64), np.float64)
    for v, (vr, vi) in enumerate(variants):
        w[0:32, v * 64 + 0:v * 64 + 32] = vr          # spr -> yr
        w[0:32, v * 64 + 32:v * 64 + 64] = vi         # spr -> yi
        w[32:64, v * 64 + 0:v * 64 + 32] = -vi        # spi -> yr
        w[32:64, v * 64 + 32:v * 64 + 64] = vr        # spi -> yi
    out = np.zeros((128, 5 * 64), np.float32)
    out[0:64] = w
    out[64:128] = w
    return out.astype(BF16_NP)


def _temporal_taps(t):
    if t == 0:
        return [(0, 3), (1, 2)]
    if t == T - 1:
        return [(T - 2, 0), (T - 1, 4)]
    return [(t - 1, 0), (t, 1), (t + 1, 2)]


def build_program():
    nc = bacc.Bacc(None, target_bir_lowering=False)

    xin = nc.declare_dram_parameter("xin", [24, T, ZP, XI, Y], BF16, isOutput=False)
    wsp = nc.declare_dram_parameter("wsp", [128, 6 * 64], BF16, isOutput=False)
    wtp = nc.declare_dram_parameter("wtp", [128, 5 * 64], BF16, isOutput=False)
    # out_c[tp, zb, p, fo]: p = zpar*64 + q*32 + f (q=0 yr, 1 yi);
    # fo = r*2048 + zp*1024 + d*512 + x*32 + yy; t = tp + 4r;
    # z = zb*4 + zp*2 + zpar; y = d*32 + yy. bf16: host upcasts.
    out_c = nc.declare_dram_parameter("out_c", [4, NZB, 128, 4096], BF16, isOutput=True)

    with tile.TileContext(nc) as tc:
        with (
            tc.tile_pool(name="wpool", bufs=1) as wpool,
            tc.tile_pool(name="slabs", bufs=4) as slab_pool,
            tc.tile_pool(name="spairs", bufs=8) as spair_pool,
            tc.tile_pool(name="stage", bufs=6) as stage_pool,
            tc.tile_pool(name="psum", bufs=8, space="PSUM") as psum_pool,
        ):
            wsp_sb = wpool.tile([128, 6 * 64], BF16, name="wsp_sb", tag="wsp")
            wtp_sb = wpool.tile([128, 5 * 64], BF16, name="wtp_sb", tag="wtp")
            nc.sync.dma_start(out=wsp_sb[:], in_=wsp[:])
            nc.sync.dma_start(out=wtp_sb[:], in_=wtp[:])



            for zb in range(NZB):
                z0 = zb * ZB

                # ---- spatial phase: all 8 t-slices per wave (row+col
                # tiles: 4 row groups x 2 col groups of 24/12x64).
                slabs = []
                for quad in range(2):
                    slab = slab_pool.tile([128, ZI * XI * Y], BF16, name="slab", tag="slab")
                    slab_v = slab.rearrange("p (z x y) -> p z x y", z=ZI, x=XI, y=Y)
                    for g in range(4):
                        t = quad * 4 + g
                        # split by z so the first waves can start before the
                        # whole slab lands
                        nc.sync.dma_start(
                            out=slab_v[32 * g:32 * g + 24, 0:3],
                            in_=xin[:, t, z0:z0 + 3],
                        )
                        nc.sync.dma_start(
                            out=slab_v[32 * g:32 * g + 24, 3:ZI],
                            in_=xin[:, t, z0 + 3:z0 + ZI],
                        )
                    slabs.append(slab_v)

                spairs = []
                for g in range(4):
                    spairs.append(
                        spair_pool.tile([128, ZB * 2 * 512], BF16, name="sp", tag="sp")
                    )

                for z in range(ZB):
                    for j in range(2):
                        banks = []
                        for g in range(4):
                            banks.append(
                                psum_pool.tile([128, 512], F32, name="ps", tag="ps")
                            )
                        # Wave-rotated issue across all 8 tiles so LDWEIGHTS
                        # hits a different tile than the in-flight matmul
                        # (background weight buffer) and all tiles stream
                        # concurrently. The two col groups of a bank live on
                        # disjoint partition halves; interleaving their
                        # accumulation groups is HW-safe (probed), only the
                        # sim's partition-blind group check minds — skipped.
                        for w in range(6):
                            dx, grp = w // 2, w % 2
                            kk = 24 if grp == 0 else 12
                            zoff = z if grp == 0 else z + 2
                            for col in range(2):
                                for g in range(4):
                                    nc.tensor.matmul(
                                        out=banks[g][64 * col:64 * col + 64, :],
                                        lhsT=wsp_sb[
                                            32 * g:32 * g + kk,
                                            w * 64:(w + 1) * 64,
                                        ],
                                        rhs=slabs[col][
                                            32 * g:32 * g + kk,
                                            zoff,
                                            dx:dx + XC,
                                            32 * j:32 * j + 32,
                                        ],
                                        start=(w == 0),
                                        stop=(w == 5),
                                        tile_position=(32 * g, 64 * col),
                                        skip_group_check=True,
                                    )
                        # spair[g] partitions: (t//4)*64 + q*32 + f
                        # free: (z*2 + j)*512 + x*32 + yy
                        fo = (z * 2 + j) * 512
                        for g in range(4):
                            dst = spairs[g][:, fo:fo + 512]
                            if g < 2:
                                nc.scalar.copy(dst, banks[g][:])
                            else:
                                nc.vector.tensor_copy(dst, banks[g][:])

                # ---- temporal phase: 4 concurrent tiles (2 t-groups x 2 zpar)
                for tp in range(4):
                    stage = stage_pool.tile([128, 4096], BF16, name="stg", tag="stg")
                    for zp in range(2):
                        plans = []
                        allsets = []
                        for r in range(2):
                            t = tp + 4 * r
                            taps = _temporal_taps(t)
                            # A group's taps must all use the same PE tile row
                            # (changing tile_position mid-accumulation-group
                            # crashes the HW), so split taps by source row and
                            # give each row-run its own bank, summed at evac.
                            runs = []
                            for (s, v) in taps:
                                sr = 64 * (s // 4)
                                if runs and runs[-1][0] == sr:
                                    runs[-1][1].append((s, v))
                                else:
                                    runs.append((sr, [(s, v)]))
                            bsets = []
                            for _ in runs:
                                bsets.append([
                                    psum_pool.tile([128, 512], F32, name="ps", tag="ps")
                                    for _ in range(2)
                                ])
                            plan = []
                            for bidx, (sr, rtaps) in enumerate(runs):
                                for a, (s, v) in enumerate(rtaps):
                                    plan.append((sr, s, v, a == 0,
                                                 a == len(rtaps) - 1, bidx))
                            plans.append(plan)
                            allsets.append(bsets)
                        # Interleave the t-group/d/zpar streams so consecutive
                        # matmuls hit different PE tiles and stream
                        # concurrently (accumulation groups on disjoint
                        # partition halves of a bank may interleave —
                        # HW-probed; only the sim's partition-blind group
                        # check minds, hence skip_group_check).
                        for a in range(max(len(p) for p in plans)):
                            for r in range(2):
                                if a >= len(plans[r]):
                                    continue
                                sr, s, v, first, last, bidx = plans[r][a]
                                sp = spairs[s % 4]
                                for d in range(2):
                                    for zpar in range(2):
                                        fo = ((2 * zp + zpar) * 2) * 512
                                        nc.tensor.matmul(
                                            out=allsets[r][bidx][d][
                                                64 * zpar:64 * zpar + 64, :
                                            ],
                                            lhsT=wtp_sb[
                                                sr:sr + 64,
                                                v * 64:(v + 1) * 64,
                                            ],
                                            rhs=sp[
                                                sr:sr + 64,
                                                fo + d * 512:fo + (d + 1) * 512,
                                            ],
                                            start=first,
                                            stop=last,
                                            tile_position=(sr, 64 * zpar),
                                            skip_group_check=True,
                                        )
                        for r in range(2):
                            bsets = allsets[r]
                            for d in range(2):
                                fo2 = r * 2048 + zp * 1024 + d * 512
                                dst = stage[:, fo2:fo2 + 512]
                                if len(bsets) == 2:
                                    # TensorTensor reads at most one PSUM
                                    # operand: stage the short run via ACT,
                                    # then add the main bank in-place on DVE.
                                    nc.scalar.copy(dst, bsets[1][d][:])
                                    nc.vector.tensor_add(
                                        dst, bsets[0][d][:], dst
                                    )
                                elif d == 0:
                                    nc.scalar.copy(dst, bsets[0][d][:])
                                else:
                                    nc.vector.tensor_copy(dst, bsets[0][d][:])
                    # split across partition halves -> two DMA queues
                    nc.sync.dma_start(out=out_c[tp, zb, 0:64], in_=stage[0:64, :])
                    nc.sync.dma_start(out=out_c[tp, zb, 64:128], in_=stage[64:128, :])

    nc.finalize()
    return nc


def _prep_inputs(xr, xi, wxyz_r, wxyz_i, wt_r, wt_i):
    xr = np.asarray(xr, np.float32)
    xi = np.asarray(xi, np.float32)

    wsr, wsi = _project(np.asarray(wxyz_r, np.float64), np.asarray(wxyz_i, np.float64), True)
    wtr, wti = _project(np.asarray(wt_r, np.float64), np.asarray(wt_i, np.float64), False)
    wsp = _spatial_lhsT(wsr, wsi)
    wtp = _temporal_lhsT(wtr, wti)

    pads = [(0, 0), (0, 0), (1, 1), (1, 1), (1, 1), (0, 0)]
    xp = np.stack([np.pad(xr, pads, mode="symmetric"),
                   np.pad(xi, pads, mode="symmetric")])  # [2, B, T, ZP, YP, XP]
    xp = xp.astype(BF16_NP)
    in_maps = []
    for core in range(8):
        b, cx = divmod(core, NXC)
        xs = xp[:, b, :, :, :, XC * cx:XC * cx + XI, :]   # [2, T, ZP, YP, XI, C]
        blocks = []
        for dz in (0, 1):
            zi = np.minimum(np.arange(ZP) + dz, ZP - 1)
            zs = xs[:, :, zi]
            ys = np.stack([zs[:, :, :, dy:dy + Y] for dy in range(KY)], axis=1)
            blocks.append(ys)                       # [2, 3, T, ZP, Y, XI, C]
        bl = np.stack(blocks, axis=1)               # [part, dz, dy, T, ZP, Y, XI, C]
        bl = bl.transpose(1, 2, 7, 0, 3, 4, 6, 5)   # [dz, dy, c, part, T, ZP, XI, Y]
        xin = np.ascontiguousarray(bl.reshape(24, T, ZP, XI, Y))
        in_maps.append({"xin": xin, "wsp": wsp, "wtp": wtp})
    return in_maps


def kernel(xr, xi, wxyz_r, wxyz_i, wt_r, wt_i):
    if "nc" not in _NC_CACHE:
        _NC_CACHE["nc"] = build_program()
    nc = _NC_CACHE["nc"]

    in_maps = _prep_inputs(xr, xi, wxyz_r, wxyz_i, wt_r, wt_i)
    res = run_bass_kernel_spmd(nc, in_maps, list(range(8)))

    yr = np.empty((B, T, Z, Y, X, F), np.float32)
    yi = np.empty((B, T, Z, Y, X, F), np.float32)
    for core in range(8):
        b, cx = divmod(core, NXC)
        oc = np.asarray(res.results[core]["out_c"]).astype(np.float32)
        # [tp, zb, zpar(2), q(2), f(32), r(2), zp(2), d(2), x(16), yy(32)]
        arr = oc.reshape(4, NZB, 2, 2, 32, 2, 2, 2, 16, 32)
        # -> [r, tp, zb, zp, zpar, d, yy, x, f, q]; t = 4r + tp
        arr = arr.transpose(5, 0, 1, 6, 2, 7, 9, 8, 4, 3)
        yr[b, :, :, :, XC * cx:XC * cx + XC, :] = (
            arr[..., 0].reshape(T, Z, Y, XC, F)
        )
        yi[b, :, :, :, XC * cx:XC * cx + XC, :] = (
            arr[..., 1].reshape(T, Z, Y, XC, F)
        )
    return yr, yi
